# revision 20
# baseline (speedup 1.0000x reference)
"""Trainium2 Bass kernel for nn_CMFuser (topk_masking).

Self-contained: accepts FULL inputs (as produced by setup_inputs()), returns
the FULL [32, 512, 768] output. Internally shards batch across 8 NeuronCores
(pure data parallel, 4 batches/core) and runs a hand-written Bass/Tile kernel.

Algorithmic structure (validated against the jax reference to ~1e-3):
  * BN(eval) + topk-channel-exchange blend folds into per-channel affine:
        x0_rgb = A1*rgb + A2*depth + A3,   x0_depth = D1*depth + D2*rgb + D3
  * The 2-token attention with -1e9 diag mask is an EXACT token swap
    (exp(-1e9) underflows to 0 in f32), so qkv+softmax+proj collapse into
    one fused C x C matmul Wc = proj_w @ Wv applied to the OTHER token.
  * LN weights fold into the following matmul; LN mean-subtraction folds
    into a rank-1 (K=1) matmul correction on the output.
  * Final LN + mean over the 2 modality tokens folds into 0.5*wf scale.

Device layout: channel-major activations [128 channels, 512 tokens] per tile;
LN statistics via ones-matmuls on the PE; per-token broadcast via K=1 matmuls.
Matmuls for Wc/fc1/fc2 run in bf16 (weights + normalized activations);
everything else (residual stream, statistics) stays f32 / f32r.
"""

import os
import sys

sys.path.insert(0, "/opt/trn_rl_repo")

import numpy as np
import ml_dtypes

import concourse.bass as bass
import concourse.mybir as mybir
import concourse.tile as tile
from concourse.bass_utils import run_bass_kernel_spmd
from contextlib import ExitStack

dt = mybir.dt
Alu = mybir.AluOpType
Act = mybir.ActivationFunctionType

B, T, C = 32, 512, 768
H = 4
K_EX = int(C * 0.2)
MLP = 4 * C
EPS = 1e-5
N_CORES = 8
B_CORE = B // N_CORES          # 4 batches per core
ROWS = B_CORE * T              # 2048 token-sites per core
TG = 512                       # tokens per group (= T)
NG = ROWS // TG                # 4 groups per core
CT = C // 128                  # 6 channel tiles
MT = MLP // 128                # 24 mlp tiles
NTT = TG // 128                # 4 token tiles per group

# vector slot indices in the packed per-channel constant table
V_A1, V_A2, V_A3, V_D1, V_D2, V_D3, V_PB, V_FC2B, V_WFH, V_BF = range(10)
NV = 10

_CACHE = {}


def _build_nc(act_fn=None):
    """Build the per-core Bass module (same program on all 8 cores)."""
    if act_fn is None:
        act_fn = Act.Gelu
    nc = bass.Bass()

    rgb_d = nc.dram_tensor("rgb", [ROWS, C], dt.float32, kind="ExternalInput")
    dep_d = nc.dram_tensor("dep", [ROWS, C], dt.float32, kind="ExternalInput")
    wc_d = nc.dram_tensor("wc", [128, CT * C], dt.bfloat16, kind="ExternalInput")
    fc1_d = nc.dram_tensor("fc1", [128, CT * MLP], dt.bfloat16, kind="ExternalInput")
    fc2_d = nc.dram_tensor("fc2", [128, MT * C], dt.bfloat16, kind="ExternalInput")
    vecs_d = nc.dram_tensor("vecs", [128, CT * NV], dt.float32, kind="ExternalInput")
    fb1_d = nc.dram_tensor("fb1", [128, MT], dt.float32, kind="ExternalInput")
    wcsum_d = nc.dram_tensor("wcsum", [1, C], dt.bfloat16, kind="ExternalInput")
    fc1sum_d = nc.dram_tensor("fc1sum", [1, MLP], dt.bfloat16, kind="ExternalInput")
    ident_d = nc.dram_tensor("ident", [128, 128], dt.float32, kind="ExternalInput")
    out_d = nc.dram_tensor("out", [ROWS, C], dt.float32, kind="ExternalOutput")

    f32r = dt.float32r

    with tile.TileContext(nc) as tc, ExitStack() as ctx:
        const = ctx.enter_context(tc.tile_pool(name="const", bufs=1))
        inp = ctx.enter_context(tc.tile_pool(name="inp", bufs=8))
        resp = ctx.enter_context(tc.tile_pool(name="resp", bufs=12))
        hp = ctx.enter_context(tc.tile_pool(name="hp", bufs=12))
        sqp = ctx.enter_context(tc.tile_pool(name="sqp", bufs=2))
        xbp = ctx.enter_context(tc.tile_pool(name="xbp", bufs=3))
        tmpp = ctx.enter_context(tc.tile_pool(name="tmpp", bufs=2))
        apool = ctx.enter_context(tc.tile_pool(name="apool", bufs=2))
        rows2 = ctx.enter_context(tc.tile_pool(name="rows2", bufs=6))
        rows1 = ctx.enter_context(tc.tile_pool(name="rows1", bufs=6))
        uaffp = ctx.enter_context(tc.tile_pool(name="uaffp", bufs=6))
        outp = ctx.enter_context(tc.tile_pool(name="outp", bufs=1))
        psum = ctx.enter_context(
            tc.tile_pool(name="psum", bufs=2, space="PSUM")
        )

        # ---- constants / weights ----
        wc_sb = const.tile([128, CT * C], dt.bfloat16)
        nc.sync.dma_start(wc_sb[:], wc_d[:])
        fc1_sb = const.tile([128, CT * MLP], dt.bfloat16)
        nc.sync.dma_start(fc1_sb[:], fc1_d[:])
        fc2_sb = const.tile([128, MT * C], dt.bfloat16)
        nc.sync.dma_start(fc2_sb[:], fc2_d[:])
        vecs_sb = const.tile([128, CT * NV], dt.float32)
        nc.sync.dma_start(vecs_sb[:], vecs_d[:])
        fb1_sb = const.tile([128, MT], dt.float32)
        nc.sync.dma_start(fb1_sb[:], fb1_d[:])
        wcsum_sb = const.tile([1, C], dt.bfloat16)
        nc.sync.dma_start(wcsum_sb[:], wcsum_d[:])
        fc1sum_sb = const.tile([1, MLP], dt.bfloat16)
        nc.sync.dma_start(fc1sum_sb[:], fc1sum_d[:])
        ident_sb = const.tile([128, 128], dt.float32)
        nc.sync.dma_start(ident_sb[:], ident_d[:])
        # PE matmuls can carry only one sync wait; HWDGE DMAs may split
        # across queues (multiple semaphores). Interpose a no-op compute
        # touch on every DMA-produced tensor the PE reads directly.
        for _t in (wc_sb, fc1_sb, fc2_sb, wcsum_sb, fc1sum_sb, ident_sb,
                   vecs_sb, fb1_sb):
            nc.scalar.copy(_t[:], _t[:])
        # Each ISA instruction carries at most ONE sync wait. Make the other
        # engines observe the ACT guard-copy clock once, up front, so later
        # per-instruction waits collapse to a single new semaphore.
        obs = const.tile([1, 4], dt.float32)
        nc.vector.tensor_copy(obs[0:1, 0:1], vecs_sb[0:1, 0:1])
        nc.gpsimd.memset(obs[0:1, 2:3], 0.0)

        ones_row = const.tile([1, 128], dt.float32)
        nc.vector.memset(ones_row[:], 1.0)
        ones_col = const.tile([128, 1], dt.bfloat16)
        nc.vector.memset(ones_col[:], 1.0)
        ones_row_b = const.tile([1, 128], dt.bfloat16)
        nc.vector.memset(ones_row_b[:], 1.0)

        def vec(idx, j):
            # per-channel scalar [128,1] for channel tile j
            return vecs_sb[:, j * NV + idx : j * NV + idx + 1]

        def ln_stats(xr, xd, name):
            """LN stats over the channel dim, per stream.

            xr/xd: lists of 6 [128,512] f32 SBUF tiles (channel-major).
            Returns dict with [1,512] SBUF rows: r_r, r_d (rsqrt) and
            mr_r, mr_d (mean*rsqrt).
            """
            out = {}
            for s, tiles in ((0, xr), (1, xd)):
                sfx = "r" if s == 0 else "d"
                xb = []
                sq = []
                for j in range(CT):
                    xbt = xbp.tile([128, TG], dt.bfloat16, tag="xb",
                                   name=f"xb_{name}_{s}_{j}")
                    nc.vector.tensor_copy(xbt[:], tiles[j][:])
                    xb.append(xbt)
                    sqt = sqp.tile([128, TG], dt.bfloat16, tag="sq",
                                   name=f"sq_{name}_{s}_{j}")
                    nc.scalar.square(sqt[:], xbt[:])
                    sq.append(sqt)
                psx = psum.tile([128, TG], dt.float32, tag="ps",
                                name=f"psx_{name}_{s}")
                psx2 = psum.tile([128, TG], dt.float32, tag="ps",
                                 name=f"psx2_{name}_{s}")
                for j in range(CT):
                    nc.tensor.matmul(psx[0:1, :], ones_col[:],
                                     xb[j][:],
                                     start=(j == 0), stop=(j == CT - 1))
                for j in range(CT):
                    nc.tensor.matmul(psx2[0:1, :], ones_col[:],
                                     sq[j][:],
                                     start=(j == 0), stop=(j == CT - 1))
                mrow = rows2.tile([1, TG], dt.float32, tag="rows",
                                  name=f"m_{name}_{s}")
                nc.vector.tensor_scalar(mrow[:], psx[0:1, :], 1.0 / C, None,
                                        Alu.mult)
                ex2 = rows2.tile([1, TG], dt.float32, tag="rows",
                                 name=f"ex2_{name}_{s}")
                nc.vector.tensor_scalar(ex2[:], psx2[0:1, :], 1.0 / C, None,
                                        Alu.mult)
                msq = rows2.tile([1, TG], dt.float32, tag="rows",
                                 name=f"msq_{name}_{s}")
                nc.scalar.square(msq[:], mrow[:])
                var = rows2.tile([1, TG], dt.float32, tag="rows",
                                 name=f"var_{name}_{s}")
                # var + eps = (ex2 + eps) - m^2, fused in one op
                nc.vector.scalar_tensor_tensor(var[:], ex2[:], EPS, msq[:],
                                               Alu.add, Alu.subtract)
                std = rows2.tile([1, TG], dt.float32, tag="rows",
                                 name=f"std_{name}_{s}")
                nc.scalar.sqrt(std[:], var[:])
                rrow = rows1.tile([1, TG], dt.float32, tag="rows1",
                                  name=f"r_{name}_{s}")
                nc.vector.reciprocal(rrow[:], std[:])
                mr = rows1.tile([1, TG], dt.bfloat16, tag="rows1b",
                                name=f"mr_{name}_{s}")
                nc.vector.tensor_tensor(mr[:], mrow[:], rrow[:], Alu.mult)
                out[f"r_{sfx}"] = rrow[:]
                out[f"mr_{sfx}"] = mr[:]
            return out

        def bcast(row_ap, name, tag="ps"):
            """Broadcast a [1,512] SBUF row across 128 partitions via K=1 MM."""
            bc = psum.tile([128, TG], dt.float32, tag=tag, name=f"bc_{name}")
            nc.tensor.matmul(bc[:], ones_row[0:1, :],
                             row_ap, start=True, stop=True)
            return bc

        # ================= main loop over groups =================
        for g in range(NG):
            r0 = g * TG
            # ---- stage L: load token-major, PE-transpose, blend ----
            in_tiles = {}
            for s, src in ((0, rgb_d), (1, dep_d)):
                for tt in range(NTT):
                    it = inp.tile([128, C], dt.float32, tag="in",
                                  name=f"in_{g}_{s}_{tt}")
                    nc.sync.dma_start(
                        it[:], src[r0 + tt * 128 : r0 + (tt + 1) * 128, :])
                    nc.scalar.copy(it[:], it[:])
                    in_tiles[s, tt] = it
            x = {}          # (s, j) -> [128, TG] f32 residual tiles
            for j in range(CT):
                pt = {}
                for s in (0, 1):
                    p = psum.tile([128, TG], dt.float32, tag="ps",
                                  name=f"pt_{g}_{s}_{j}")
                    for tt in range(NTT):
                        nc.tensor.transpose(
                            p[:, tt * 128 : (tt + 1) * 128],
                            in_tiles[s, tt][:, j * 128 : (j + 1) * 128],
                            ident_sb[:])
                    pt[s] = p
                t1 = tmpp.tile([128, TG], dt.float32, tag="bl",
                               name=f"t1_{g}_{j}")
                nc.vector.tensor_scalar(t1[:], pt[1][:], vec(V_A2, j),
                                        vec(V_A3, j), Alu.mult, Alu.add)
                x0r = resp.tile([128, TG], dt.float32, tag="res",
                                name=f"x0r_{g}_{j}")
                nc.vector.scalar_tensor_tensor(x0r[:], pt[0][:], vec(V_A1, j),
                                               t1[:], Alu.mult, Alu.add)
                t2 = tmpp.tile([128, TG], dt.float32, tag="bl",
                               name=f"t2_{g}_{j}")
                nc.vector.tensor_scalar(t2[:], pt[0][:], vec(V_D2, j),
                                        vec(V_D3, j), Alu.mult, Alu.add)
                x0d = resp.tile([128, TG], dt.float32, tag="res",
                                name=f"x0d_{g}_{j}")
                nc.vector.scalar_tensor_tensor(x0d[:], pt[1][:], vec(V_D1, j),
                                               t2[:], Alu.mult, Alu.add)
                x[0, j] = x0r
                x[1, j] = x0d

            # ---- norm1 + attention (exact swap) ----
            st1 = ln_stats([x[0, j] for j in range(CT)],
                           [x[1, j] for j in range(CT)], f"n1_{g}")
            h = {}
            for s in (0, 1):
                bc = bcast(st1["r_r" if s == 0 else "r_d"], f"n1_{g}_{s}")
                for j in range(CT):
                    ht = hp.tile([128, TG], dt.bfloat16, tag="h",
                                 name=f"h1_{g}_{s}_{j}")
                    nc.vector.tensor_tensor(ht[:], x[s, j][:], bc[0:128, :],
                                            Alu.mult)
                    h[s, j] = ht
            # per mo: g_s then immediately the swapped residual
            for mo in range(CT):
                gps = {}
                for s in (0, 1):
                    pg = psum.tile([128, TG], dt.float32, tag="ps",
                                   name=f"g_{g}_{s}_{mo}")
                    for k in range(CT):
                        nc.tensor.matmul(
                            pg[:],
                            wc_sb[:, k * C + mo * 128 : k * C + (mo + 1) * 128],
                            h[s, k][:], start=(k == 0), stop=False)
                    mr = st1["mr_r" if s == 0 else "mr_d"]
                    nc.tensor.matmul(
                        pg[:],
                        wcsum_sb[0:1, mo * 128 : (mo + 1) * 128],
                        mr, start=False, stop=True)
                    gps[s] = pg
                # x1_r = x0_r + g_d + pb ;  x1_d = x0_d + g_r + pb  (in place)
                nc.vector.scalar_tensor_tensor(x[0, mo][:], gps[1][:],
                                               vec(V_PB, mo), x[0, mo][:],
                                               Alu.add, Alu.add)
                nc.vector.scalar_tensor_tensor(x[1, mo][:], gps[0][:],
                                               vec(V_PB, mo), x[1, mo][:],
                                               Alu.add, Alu.add)

            # ---- norm2 + MLP ----
            st2 = ln_stats([x[0, j] for j in range(CT)],
                           [x[1, j] for j in range(CT)], f"n2_{g}")
            h2 = {}
            for s in (0, 1):
                bc = bcast(st2["r_r" if s == 0 else "r_d"], f"n2_{g}_{s}")
                for j in range(CT):
                    ht = hp.tile([128, TG], dt.bfloat16, tag="h",
                                 name=f"h2_{g}_{s}_{j}")
                    nc.vector.tensor_tensor(ht[:], x[s, j][:], bc[0:128, :],
                                            Alu.mult)
                    h2[s, j] = ht
            for s in (0, 1):
                mr2 = st2["mr_r" if s == 0 else "mr_d"]
                acc = []
                for co in range(CT):
                    a_ = psum.tile([128, TG], dt.float32, tag="acc", bufs=6,
                                   name=f"acc_{g}_{s}_{co}")
                    acc.append(a_)
                for m in range(MT):
                    pf = psum.tile([128, TG], dt.float32, tag="ps",
                                   name=f"pf_{g}_{s}_{m}")
                    for k in range(CT):
                        nc.tensor.matmul(
                            pf[:],
                            fc1_sb[:, k * MLP + m * 128 : k * MLP + (m + 1) * 128],
                            h2[s, k][:], start=(k == 0), stop=False)
                    nc.tensor.matmul(
                        pf[:],
                        fc1sum_sb[0:1, m * 128 : (m + 1) * 128],
                        mr2, start=False, stop=True)
                    am = apool.tile([128, TG], dt.bfloat16, tag="a",
                                    name=f"a_{g}_{s}_{m}")
                    nc.scalar.activation(am[:], pf[:], act_fn,
                                         bias=fb1_sb[:, m : m + 1], scale=1.0)
                    for co in range(CT):
                        nc.tensor.matmul(
                            acc[co][:],
                            fc2_sb[:, m * C + co * 128 : m * C + (co + 1) * 128],
                            am[:], start=(m == 0), stop=(m == MT - 1))
                for co in range(CT):
                    nc.vector.scalar_tensor_tensor(x[s, co][:], acc[co][:],
                                                   vec(V_FC2B, co), x[s, co][:],
                                                   Alu.add, Alu.add)

            # ---- final norm + modality mean + transpose out ----
            stf = ln_stats([x[0, j] for j in range(CT)],
                           [x[1, j] for j in range(CT)], f"nf_{g}")
            bc_rr = bcast(stf["r_r"], f"nf_{g}_r")
            bc_rd = bcast(stf["r_d"], f"nf_{g}_d")
            # broadcast of (mr_r + mr_d): two accumulated K=1 ones matmuls
            bc_mrs = psum.tile([128, TG], dt.float32, tag="acc", bufs=6,
                               name=f"bcmrs_{g}")
            nc.tensor.matmul(bc_mrs[:], ones_row_b[:],
                             stf["mr_r"], start=True, stop=False)
            nc.tensor.matmul(bc_mrs[:], ones_row_b[:],
                             stf["mr_d"], start=False, stop=True)
            uas = []
            for j in range(CT):
                s1 = tmpp.tile([128, TG], dt.float32, tag="bl",
                               name=f"nf1_{g}_{j}")
                nc.vector.tensor_tensor(s1[:], x[0, j][:], bc_rr[0:128, :],
                                        Alu.mult)
                s2 = tmpp.tile([128, TG], dt.float32, tag="bl",
                               name=f"nf2_{g}_{j}")
                nc.vector.tensor_tensor(s2[:], x[1, j][:], bc_rd[0:128, :],
                                        Alu.mult)
                nc.vector.tensor_tensor(s1[:], s1[:], s2[:], Alu.add)
                nc.vector.tensor_tensor(s1[:], s1[:], bc_mrs[0:128, :],
                                        Alu.subtract)
                ua = uaffp.tile([128, TG], dt.float32, tag="uaff",
                                name=f"ua_{g}_{j}")
                nc.scalar.activation(ua[:], s1[:], Act.Identity,
                                     bias=vec(V_BF, j), scale=vec(V_WFH, j))
                uas.append(ua)
            for tt in range(NTT):
                po = psum.tile([128, TG], dt.float32, tag="ps",
                               name=f"po_{g}_{tt}")
                po2 = psum.tile([128, TG], dt.float32, tag="ps",
                                name=f"po2_{g}_{tt}")
                for j in range(CT):
                    dst = (po[:, j * 128 : (j + 1) * 128] if j < 4
                           else po2[:, (j - 4) * 128 : (j - 3) * 128])
                    nc.tensor.transpose(
                        dst, uas[j][:, tt * 128 : (tt + 1) * 128], ident_sb[:])
                ot = outp.tile([128, C], dt.float32, tag="ot",
                               name=f"ot_{g}_{tt}")
                nc.scalar.copy(ot[:, 0:512], po[:, :])
                nc.scalar.copy(ot[:, 512:768], po2[:, 0:256])
                nc.sync.dma_start(
                    out_d[r0 + tt * 128 : r0 + (tt + 1) * 128, :], ot[:])

    _legalize_waits(nc)
    nc.finalize()
    return nc


def _legalize_waits(nc):
    """Walrus ISA structs have at most 1-2 sync-wait slots per instruction,
    but Tile's wait assignment can emit more. Move excess waits onto
    same-engine NoOps inserted immediately before the offending instruction
    (engines execute their stream in order, so an earlier wait on the same
    engine is equivalent)."""
    import bass_rust
    nop_i = [0]
    for f in nc.m.functions:
        for b in f.blocks:
            insts = b.instructions
            out = []
            changed = False
            for ins in insts:
                si = getattr(ins, "sync_info", None)
                waits = list(si.on_wait) if (si and si.on_wait) else []
                if len(waits) > 1:
                    eng = ins.engine
                    for w in waits[:-1]:
                        n = bass_rust.InstNoOp(name=f"I-nopw-{nop_i[0]}")
                        nop_i[0] += 1
                        n.engine = eng
                        n.sync_info = bass_rust.SyncInfo(
                            on_wait=[w], on_update=[])
                        out.append(n)
                    ins.sync_info = bass_rust.SyncInfo(
                        on_wait=[waits[-1]], on_update=list(si.on_update or []))
                    changed = True
                out.append(ins)
            if changed:
                b.instructions = out


def _prepare(inputs):
    """Host-side folding: per-channel vectors + fused/packed weights."""
    f = lambda k: np.asarray(inputs[k], np.float64)
    alpha = f("alpha").reshape(C)

    s_r = f("bn_rgb_w") / np.sqrt(f("bn_rgb_var") + EPS)
    t_r = f("bn_rgb_b") - f("bn_rgb_mean") * s_r
    s_d = f("bn_depth_w") / np.sqrt(f("bn_depth_var") + EPS)
    t_d = f("bn_depth_b") - f("bn_depth_mean") * s_d

    w_r = np.asarray(inputs["bn_rgb_w"], np.float32)
    w_d = np.asarray(inputs["bn_depth_w"], np.float32)
    idx_r = np.argsort(np.abs(w_r), kind="stable")[:K_EX]
    idx_d = np.argsort(np.abs(w_d), kind="stable")[:K_EX]
    mask_r = np.zeros(C, bool)
    mask_r[idx_r] = True
    mask_d = np.zeros(C, bool)
    mask_d[idx_d] = True

    A1 = np.where(mask_r, alpha * s_r, s_r)
    A2 = np.where(mask_r, (1 - alpha) * s_d, 0.0)
    A3 = np.where(mask_r, alpha * t_r + (1 - alpha) * t_d, t_r)
    D1 = np.where(mask_d, alpha * s_d, s_d)
    D2 = np.where(mask_d, (1 - alpha) * s_r, 0.0)
    D3 = np.where(mask_d, alpha * t_d + (1 - alpha) * t_r, t_d)

    qkv_w = f("qkv_w")
    Wv = qkv_w[2 * C :, :]
    Wc = f("proj_w") @ Wv
    w1, b1 = f("norm1_w"), f("norm1_b")
    Wc_f = Wc * w1[None, :]
    pb = f("proj_b") + Wc @ b1
    wc_rowsum = Wc_f.sum(axis=1)

    w2, b2 = f("norm2_w"), f("norm2_b")
    fc1_f = f("fc1_w") * w2[None, :]
    fb1 = f("fc1_b") + f("fc1_w") @ b2
    fc1_rowsum = fc1_f.sum(axis=1)
    fc2_w = f("fc2_w")
    fc2_b = f("fc2_b")
    wfh = 0.5 * f("normf_w")
    bf_ = f("normf_b")

    bf16 = ml_dtypes.bfloat16

    def pack_lhsT(wT, kt, m):
        # wT: [kt*128, m]  ->  [128, kt*m] with [p, k*m + col] = wT[128k+p, col]
        return np.ascontiguousarray(
            wT.reshape(kt, 128, m).transpose(1, 0, 2).reshape(128, kt * m))

    wc_pack = pack_lhsT(np.ascontiguousarray(Wc_f.T), CT, C).astype(bf16)
    fc1_pack = pack_lhsT(np.ascontiguousarray(fc1_f.T), CT, MLP).astype(bf16)
    fc2_pack = pack_lhsT(np.ascontiguousarray(fc2_w.T), MT, C).astype(bf16)

    vv = [A1, A2, A3, D1, D2, D3, pb, fc2_b, wfh, bf_]
    vecs = np.stack(vv, axis=-1).astype(np.float32)          # [C, NV]
    vecs = vecs.reshape(CT, 128, NV).transpose(1, 0, 2).reshape(128, CT * NV)
    vecs = np.ascontiguousarray(vecs)
    fb1_pack = np.ascontiguousarray(
        fb1.astype(np.float32).reshape(MT, 128).T)           # [128, MT]

    return {
        "wc": wc_pack,
        "fc1": fc1_pack,
        "fc2": fc2_pack,
        "vecs": vecs,
        "fb1": fb1_pack,
        "wcsum": (-wc_rowsum).astype(bf16).reshape(1, C),
        "fc1sum": (-fc1_rowsum).astype(bf16).reshape(1, MLP),
        "ident": np.eye(128, dtype=np.float32),
    }


def kernel(**inputs) -> np.ndarray:
    rgb = np.ascontiguousarray(np.asarray(inputs["rgb"], np.float32))
    dep = np.ascontiguousarray(np.asarray(inputs["depth"], np.float32))
    consts = _prepare(inputs)

    if "nc" not in _CACHE:
        _CACHE["nc"] = _build_nc()
    nc = _CACHE["nc"]

    in_maps = []
    for i in range(N_CORES):
        m = {
            "rgb": rgb[i * B_CORE : (i + 1) * B_CORE].reshape(ROWS, C),
            "dep": dep[i * B_CORE : (i + 1) * B_CORE].reshape(ROWS, C),
        }
        m.update(consts)
        in_maps.append(m)

    trace = bool(int(os.environ.get("KERNEL_TRACE", "0")))
    res = run_bass_kernel_spmd(nc, in_maps, list(range(N_CORES)), trace=trace)
    _CACHE["last_results"] = res
    out = np.concatenate(
        [res.results[i]["out"].reshape(B_CORE, T, C) for i in range(N_CORES)],
        axis=0)
    return out


if __name__ == "__main__":
    rng = np.random.default_rng(0)
    demo = {
        "rgb": rng.standard_normal((B, T, C), np.float32),
        "depth": rng.standard_normal((B, T, C), np.float32),
    }
    print("built module ok" if _build_nc() else "")


# revision 33
# speedup vs baseline: 7056.1296x; 7056.1296x over previous
"""Trainium2 Bass kernel for nn_CMFuser (topk_masking).

Self-contained: accepts FULL inputs (as produced by setup_inputs()), returns
the FULL [32, 512, 768] output. Internally shards batch across 8 NeuronCores
(pure data parallel, 4 batches/core) and runs a hand-written Bass/Tile kernel.

Algorithmic structure (validated against the jax reference to ~1e-3):
  * BN(eval) + topk-channel-exchange blend folds into per-channel affine:
        x0_rgb = A1*rgb + A2*depth + A3,   x0_depth = D1*depth + D2*rgb + D3
  * The 2-token attention with -1e9 diag mask is an EXACT token swap
    (exp(-1e9) underflows to 0 in f32), so qkv+softmax+proj collapse into
    one fused C x C matmul Wc = proj_w @ Wv applied to the OTHER token.
  * LN weights fold into the following matmul; LN mean-subtraction folds
    into a rank-1 (K=1) matmul correction on the output.
  * Final LN + mean over the 2 modality tokens folds into 0.5*wf scale.

Device layout: channel-major activations [128 channels, 512 tokens] per tile;
LN statistics via ones-matmuls on the PE; per-token broadcast via K=1 matmuls.
Matmuls for Wc/fc1/fc2 run in bf16 (weights + normalized activations);
everything else (residual stream, statistics) stays f32 / f32r.
"""

import os
import sys

sys.path.insert(0, "/opt/trn_rl_repo")

import numpy as np
import ml_dtypes

import concourse.bass as bass
import concourse.mybir as mybir
import concourse.tile as tile
from concourse.bass_utils import run_bass_kernel_spmd
from contextlib import ExitStack

dt = mybir.dt
Alu = mybir.AluOpType
Act = mybir.ActivationFunctionType

B, T, C = 32, 512, 768
H = 4
K_EX = int(C * 0.2)
MLP = 4 * C
EPS = 1e-5
N_CORES = 8
B_CORE = B // N_CORES          # 4 batches per core
ROWS = B_CORE * T              # 2048 token-sites per core
TG = 512                       # tokens per group (= T)
NG = ROWS // TG                # groups per core
CT = C // 128                  # 6 channel tiles
MT = MLP // 128                # 24 mlp tiles
NTT = TG // 128                # 4 token tiles per group

# vector slot indices in the packed per-channel constant table
V_A1, V_A2, V_A3, V_D1, V_D2, V_D3, V_PB, V_FC2B, V_WFH, V_BF = range(10)
NV = 10

_CACHE = {}


def _build_nc(act_fn=None, n_groups=NG, legalize=True):
    """Build the per-core Bass module (same program on all 8 cores)."""
    if act_fn is None:
        act_fn = Act.Gelu
    nc = bass.Bass()

    rgb_d = nc.dram_tensor("rgb", [ROWS, C], dt.float32, kind="ExternalInput")
    dep_d = nc.dram_tensor("dep", [ROWS, C], dt.float32, kind="ExternalInput")
    wc_d = nc.dram_tensor("wc", [128, CT * C], dt.bfloat16, kind="ExternalInput")
    fc1_d = nc.dram_tensor("fc1", [128, CT * MLP], dt.bfloat16, kind="ExternalInput")
    fc2_d = nc.dram_tensor("fc2", [128, MT * C], dt.bfloat16, kind="ExternalInput")
    vecs_d = nc.dram_tensor("vecs", [128, CT * NV], dt.float32, kind="ExternalInput")
    fb1_d = nc.dram_tensor("fb1", [128, MT], dt.float32, kind="ExternalInput")
    wcsum_d = nc.dram_tensor("wcsum", [1, C], dt.bfloat16, kind="ExternalInput")
    fc1sum_d = nc.dram_tensor("fc1sum", [1, MLP], dt.bfloat16, kind="ExternalInput")
    ident_d = nc.dram_tensor("ident", [128, 128], dt.float32, kind="ExternalInput")
    out_d = nc.dram_tensor("out", [ROWS, C], dt.float32, kind="ExternalOutput")

    f32r = dt.float32r

    with tile.TileContext(nc) as tc, ExitStack() as ctx:
        const = ctx.enter_context(tc.tile_pool(name="const", bufs=1))
        inp = ctx.enter_context(tc.tile_pool(name="inp", bufs=8))
        resp = ctx.enter_context(tc.tile_pool(name="resp", bufs=16 if TG == 512 else 26))
        hp = ctx.enter_context(tc.tile_pool(name="hp", bufs=12 if TG == 512 else 24))
        sqp = ctx.enter_context(tc.tile_pool(name="sqp", bufs=2 if TG == 512 else 4))
        xbp = ctx.enter_context(tc.tile_pool(name="xbp", bufs=3 if TG == 512 else 5))
        tmpp = ctx.enter_context(tc.tile_pool(name="tmpp", bufs=2 if TG == 512 else 4))
        apool = ctx.enter_context(tc.tile_pool(name="apool", bufs=2 if TG == 512 else 3))
        rows2 = ctx.enter_context(tc.tile_pool(name="rows2", bufs=4))
        rows1 = ctx.enter_context(tc.tile_pool(name="rows1", bufs=4))
        uaffp = ctx.enter_context(tc.tile_pool(name="uaffp", bufs=6))
        outp = ctx.enter_context(tc.tile_pool(name="outp", bufs=1 if TG == 512 else 2))
        psum = ctx.enter_context(
            tc.tile_pool(name="psum", bufs=2, space="PSUM")
        )

        # ---- constants / weights (small, early-needed tensors first) ----
        ident_sb = const.tile([128, 128], dt.float32)
        nc.sync.dma_start(ident_sb[:], ident_d[:])
        vecs_sb = const.tile([128, CT * NV], dt.float32)
        nc.sync.dma_start(vecs_sb[:], vecs_d[:])
        fb1_sb = const.tile([128, MT], dt.float32)
        nc.sync.dma_start(fb1_sb[:], fb1_d[:])
        wcsum_sb = const.tile([1, C], dt.bfloat16)
        nc.sync.dma_start(wcsum_sb[:], wcsum_d[:])
        fc1sum_sb = const.tile([1, MLP], dt.bfloat16)
        nc.sync.dma_start(fc1sum_sb[:], fc1sum_d[:])
        # group-0 inputs issued BEFORE the big weight loads so the first
        # transposes are not stuck behind ~14MB of weight DMA in the queues
        pre_in = {}
        for s_, src_ in ((0, rgb_d), (1, dep_d)):
            for tt_ in range(NTT):
                it_ = inp.tile([128, C], dt.float32, tag="in",
                               name=f"in_0_{s_}_{tt_}")
                nc.sync.dma_start(
                    it_[:], src_[tt_ * 128 : (tt_ + 1) * 128, :])
                nc.scalar.copy(it_[:], it_[:])
                pre_in[s_, tt_] = it_
        wc_sb = const.tile([128, CT * C], dt.bfloat16)
        nc.sync.dma_start(wc_sb[:], wc_d[:])
        fc1_sb = const.tile([128, CT * MLP], dt.bfloat16)
        nc.sync.dma_start(fc1_sb[:], fc1_d[:])
        fc2_sb = const.tile([128, MT * C], dt.bfloat16)
        nc.sync.dma_start(fc2_sb[:], fc2_d[:])
        # PE matmuls can carry only one sync wait; HWDGE DMAs may split
        # across queues (multiple semaphores). Interpose a no-op compute
        # touch on every DMA-produced tensor the PE reads directly.
        for _t in (wc_sb, fc1_sb, fc2_sb, wcsum_sb, fc1sum_sb, ident_sb,
                   vecs_sb, fb1_sb):
            nc.scalar.copy(_t[:], _t[:])
        # Each ISA instruction carries at most ONE sync wait. Make the other
        # engines observe the ACT guard-copy clock once, up front, so later
        # per-instruction waits collapse to a single new semaphore.
        obs = const.tile([1, 4], dt.float32)
        nc.vector.tensor_copy(obs[0:1, 0:1], vecs_sb[0:1, 0:1])
        nc.gpsimd.memset(obs[0:1, 2:3], 0.0)

        ones_col = const.tile([128, 1], dt.bfloat16)
        nc.vector.memset(ones_col[:], 1.0)
        # bcast lhsT rows with folded sqrt(C) scaling (see ln_stats)
        sqrtc_f32 = const.tile([1, 128], dt.float32)
        nc.vector.memset(sqrtc_f32[:], float(np.sqrt(C)))
        sqrtc_row = const.tile([1, 128], dt.float32r)
        with nc.allow_low_precision("fp32r bcast lhsT"):
            nc.vector.tensor_copy(sqrtc_row[:], sqrtc_f32[:])
        isqrtc_row_b = const.tile([1, 128], dt.bfloat16)
        nc.vector.memset(isqrtc_row_b[:], float(1.0 / np.sqrt(C)))
        ceps_ap = const.tile([1, 1], dt.float32)
        nc.vector.memset(ceps_ap[:], float(C * EPS))

        def vec(idx, j):
            # per-channel scalar [128,1] for channel tile j
            return vecs_sb[:, j * NV + idx : j * NV + idx + 1]

        def ln_stats(xr, xd, name):
            """LN stats over the channel dim, per stream.

            xr/xd: lists of 6 [128,512] f32 SBUF tiles (channel-major).
            Returns dict with [1,512] SBUF rows: r_r, r_d (rsqrt) and
            mr_r, mr_d (mean*rsqrt).
            """
            out = {}
            for s, tiles in ((0, xr), (1, xd)):
                sfx = "r" if s == 0 else "d"
                xb = []
                sq = []
                for j in range(CT):
                    xbt = xbp.tile([128, TG], dt.bfloat16, tag="xb",
                                   name=f"xb_{name}_{s}_{j}")
                    nc.gpsimd.tensor_copy(xbt[:], tiles[j][:])
                    xb.append(xbt)
                    sqt = sqp.tile([128, TG], dt.bfloat16, tag="sq",
                                   name=f"sq_{name}_{s}_{j}")
                    nc.scalar.square(sqt[:], xbt[:])
                    sq.append(sqt)
                stat = psum.tile([128, TG], dt.float32, tag="ps",
                                 name=f"stat_{name}_{s}")
                # sum(x) accumulates at partition 0 (col-group 0) while
                # sum(x^2) accumulates at partition 32 (col-group 1); the
                # two M=1 matmul chains share the PE via col tiling.
                for j in range(CT):
                    nc.tensor.matmul(stat[0:1, :], ones_col[:],
                                     xb[j][:], tile_position=(0, 0),
                                     start=(j == 0), stop=(j == CT - 1))
                    nc.tensor.matmul(stat[32:33, :], ones_col[:],
                                     sq[j][:], tile_position=(0, 32),
                                     start=(j == 0), stop=(j == CT - 1))
                # With S1 = sum(x), S2 = sum(x^2):
                #   rr  = 1/sqrt(S2 - S1^2/C + C*eps) = rsqrt(var+eps)/sqrt(C)
                #   bcast of r uses a sqrt(C)-valued lhsT row;
                #   mr' = S1*rr = m*r*sqrt(C); the 1/sqrt(C) is folded into
                #   the wcsum/fc1sum correction rows host-side.
                sq1 = rows2.tile([1, TG], dt.float32, tag="rows",
                                 name=f"sq1_{name}_{s}")
                nc.scalar.square(sq1[:], stat[0:1, :])
                u = rows2.tile([1, TG], dt.float32, tag="rows",
                               name=f"u_{name}_{s}")
                nc.vector.scalar_tensor_tensor(u[:], sq1[:], -1.0 / C,
                                               stat[32:33, :], Alu.mult, Alu.add)
                std = rows2.tile([1, TG], dt.float32, tag="rows",
                                 name=f"std_{name}_{s}")
                nc.scalar.activation(std[:], u[:], Act.Sqrt,
                                     bias=ceps_ap[0:1, 0:1], scale=1.0)
                rrow = rows1.tile([1, TG], dt.float32r, tag="rows1", bufs=4,
                                  name=f"r_{name}_{s}")
                with nc.allow_low_precision("fp32r bcast rows"):
                    nc.vector.reciprocal(rrow[:], std[:])
                mr = rows1.tile([1, TG], dt.bfloat16, tag="rows1b", bufs=4,
                                name=f"mr_{name}_{s}")
                nc.vector.tensor_tensor(mr[:], stat[0:1, :], rrow[:], Alu.mult)
                out[f"r_{sfx}"] = rrow[:]
                out[f"mr_{sfx}"] = mr[:]
            return out

        def bcast(row_ap, name, tag="ps"):
            """Broadcast a [1,512] SBUF row across 128 partitions via K=1 MM."""
            bc = psum.tile([128, TG], dt.float32, tag=tag, name=f"bc_{name}")
            nc.tensor.matmul(bc[:], sqrtc_row[0:1, :],
                             row_ap, start=True, stop=True)
            return bc

        # ================= main loop over groups =================
        for g in range(n_groups):
            r0 = g * TG
            # ---- stage L: load token-major, PE-transpose, blend ----
            if g == 0:
                in_tiles = pre_in
            else:
                in_tiles = {}
                for s, src in ((0, rgb_d), (1, dep_d)):
                    for tt in range(NTT):
                        it = inp.tile([128, C], dt.float32, tag="in",
                                      name=f"in_{g}_{s}_{tt}")
                        nc.sync.dma_start(
                            it[:], src[r0 + tt * 128 : r0 + (tt + 1) * 128, :])
                        nc.scalar.copy(it[:], it[:])
                        in_tiles[s, tt] = it
            x = {}          # (s, j) -> [128, TG] f32 residual tiles
            for j in range(CT):
                pt = {}
                for s in (0, 1):
                    p = psum.tile([128, TG], dt.float32, tag="ps",
                                  name=f"pt_{g}_{s}_{j}")
                    for tt in range(NTT):
                        nc.tensor.transpose(
                            p[:, tt * 128 : (tt + 1) * 128],
                            in_tiles[s, tt][:, j * 128 : (j + 1) * 128],
                            ident_sb[:])
                    pt[s] = p
                t1 = tmpp.tile([128, TG], dt.float32, tag="bl",
                               name=f"t1_{g}_{j}")
                nc.vector.tensor_scalar(t1[:], pt[1][:], vec(V_A2, j),
                                        vec(V_A3, j), Alu.mult, Alu.add)
                x0r = resp.tile([128, TG], dt.float32, tag="res",
                                name=f"x0r_{g}_{j}")
                nc.vector.scalar_tensor_tensor(x0r[:], pt[0][:], vec(V_A1, j),
                                               t1[:], Alu.mult, Alu.add)
                t2 = tmpp.tile([128, TG], dt.float32, tag="bl",
                               name=f"t2_{g}_{j}")
                nc.vector.tensor_scalar(t2[:], pt[0][:], vec(V_D2, j),
                                        vec(V_D3, j), Alu.mult, Alu.add)
                x0d = resp.tile([128, TG], dt.float32, tag="res",
                                name=f"x0d_{g}_{j}")
                nc.vector.scalar_tensor_tensor(x0d[:], pt[1][:], vec(V_D1, j),
                                               t2[:], Alu.mult, Alu.add)
                x[0, j] = x0r
                x[1, j] = x0d

            # ---- norm1 + attention (exact swap) ----
            st1 = ln_stats([x[0, j] for j in range(CT)],
                           [x[1, j] for j in range(CT)], f"n1_{g}")
            h = {}
            for s in (0, 1):
                bc = bcast(st1["r_r" if s == 0 else "r_d"], f"n1_{g}_{s}")
                for j in range(CT):
                    ht = hp.tile([128, TG], dt.bfloat16, tag="h",
                                 name=f"h1_{g}_{s}_{j}")
                    nc.vector.tensor_tensor(ht[:], x[s, j][:], bc[0:128, :],
                                            Alu.mult)
                    h[s, j] = ht
            # k-outer into 6 psum banks: PE starts on the first h tile.
            # g_r lands first, is consumed by the x1_d residuals (swap),
            # then the banks recycle for g_d -> x1_r.
            for s, o in ((0, 1), (1, 0)):
                accs = []
                for mo in range(CT):
                    a_ = psum.tile([128, TG], dt.float32, tag="acc", bufs=6,
                                   name=f"g_{g}_{s}_{mo}")
                    accs.append(a_)
                for k in range(CT):
                    for mo in range(CT):
                        nc.tensor.matmul(
                            accs[mo][:],
                            wc_sb[:, k * C + mo * 128 : k * C + (mo + 1) * 128],
                            h[s, k][:], start=(k == 0), stop=False)
                mr = st1["mr_r" if s == 0 else "mr_d"]
                for mo in range(CT):
                    nc.tensor.matmul(
                        accs[mo][:],
                        wcsum_sb[0:1, mo * 128 : (mo + 1) * 128],
                        mr, start=False, stop=True)
                    # x1_o = x0_o + g_s + pb (in place), o = other stream
                    nc.vector.scalar_tensor_tensor(x[o, mo][:], accs[mo][:],
                                                   vec(V_PB, mo), x[o, mo][:],
                                                   Alu.add, Alu.add)

            # ---- norm2 + MLP ----
            st2 = ln_stats([x[0, j] for j in range(CT)],
                           [x[1, j] for j in range(CT)], f"n2_{g}")
            h2 = {}
            for s in (0, 1):
                bc = bcast(st2["r_r" if s == 0 else "r_d"], f"n2_{g}_{s}")
                for j in range(CT):
                    ht = hp.tile([128, TG], dt.bfloat16, tag="h",
                                 name=f"h2_{g}_{s}_{j}")
                    nc.vector.tensor_tensor(ht[:], x[s, j][:], bc[0:128, :],
                                            Alu.mult)
                    h2[s, j] = ht
            for s in (0, 1):
                mr2 = st2["mr_r" if s == 0 else "mr_d"]
                acc = []
                for co in range(CT):
                    a_ = psum.tile([128, TG], dt.float32, tag="acc", bufs=6,
                                   name=f"acc_{g}_{s}_{co}")
                    acc.append(a_)
                for m in range(MT):
                    pf = psum.tile([128, TG], dt.float32, tag="ps",
                                   name=f"pf_{g}_{s}_{m}")
                    for k in range(CT):
                        nc.tensor.matmul(
                            pf[:],
                            fc1_sb[:, k * MLP + m * 128 : k * MLP + (m + 1) * 128],
                            h2[s, k][:], start=(k == 0), stop=False)
                    nc.tensor.matmul(
                        pf[:],
                        fc1sum_sb[0:1, m * 128 : (m + 1) * 128],
                        mr2, start=False, stop=True)
                    am = apool.tile([128, TG], dt.bfloat16, tag="a",
                                    name=f"a_{g}_{s}_{m}")
                    nc.scalar.activation(am[:], pf[:], act_fn,
                                         bias=fb1_sb[:, m : m + 1], scale=1.0)
                    for co in range(CT):
                        nc.tensor.matmul(
                            acc[co][:],
                            fc2_sb[:, m * C + co * 128 : m * C + (co + 1) * 128],
                            am[:], start=(m == 0), stop=(m == MT - 1))
                for co in range(CT):
                    nc.vector.scalar_tensor_tensor(x[s, co][:], acc[co][:],
                                                   vec(V_FC2B, co), x[s, co][:],
                                                   Alu.add, Alu.add)

            # ---- final norm + modality mean + transpose out ----
            stf = ln_stats([x[0, j] for j in range(CT)],
                           [x[1, j] for j in range(CT)], f"nf_{g}")
            bc_rr = bcast(stf["r_r"], f"nf_{g}_r")
            bc_rd = bcast(stf["r_d"], f"nf_{g}_d")
            # broadcast of (mr_r + mr_d): two accumulated K=1 ones matmuls
            bc_mrs = psum.tile([128, TG], dt.float32, tag="acc", bufs=6,
                               name=f"bcmrs_{g}")
            nc.tensor.matmul(bc_mrs[:], isqrtc_row_b[:],
                             stf["mr_r"], start=True, stop=False)
            nc.tensor.matmul(bc_mrs[:], isqrtc_row_b[:],
                             stf["mr_d"], start=False, stop=True)
            uas = []
            for j in range(CT):
                s1 = tmpp.tile([128, TG], dt.float32, tag="bl",
                               name=f"nf1_{g}_{j}")
                nc.vector.tensor_tensor(s1[:], x[0, j][:], bc_rr[0:128, :],
                                        Alu.mult)
                s2 = tmpp.tile([128, TG], dt.float32, tag="bl",
                               name=f"nf2_{g}_{j}")
                nc.vector.tensor_tensor(s2[:], x[1, j][:], bc_rd[0:128, :],
                                        Alu.mult)
                nc.gpsimd.tensor_tensor(s1[:], s1[:], s2[:], Alu.add)
                nc.vector.tensor_tensor(s1[:], s1[:], bc_mrs[0:128, :],
                                        Alu.subtract)
                ua = uaffp.tile([128, TG], dt.float32, tag="uaff",
                                name=f"ua_{g}_{j}")
                nc.scalar.activation(ua[:], s1[:], Act.Identity,
                                     bias=vec(V_BF, j), scale=vec(V_WFH, j))
                uas.append(ua)
            for tt in range(NTT):
                po = psum.tile([128, 512], dt.float32, tag="acc", bufs=6,
                               name=f"po_{g}_{tt}")
                po2 = psum.tile([128, 512], dt.float32, tag="acc", bufs=6,
                                name=f"po2_{g}_{tt}")
                for j in range(CT):
                    dst = (po[:, j * 128 : (j + 1) * 128] if j < 4
                           else po2[:, (j - 4) * 128 : (j - 3) * 128])
                    nc.tensor.transpose(
                        dst, uas[j][:, tt * 128 : (tt + 1) * 128], ident_sb[:])
                ot = outp.tile([128, C], dt.float32, tag="ot",
                               name=f"ot_{g}_{tt}")
                nc.scalar.copy(ot[:, 0:512], po[:, :])
                nc.scalar.copy(ot[:, 512:768], po2[:, 0:256])
                nc.sync.dma_start(
                    out_d[r0 + tt * 128 : r0 + (tt + 1) * 128, :], ot[:])

    if legalize:
        _legalize_waits(nc)
    nc.finalize()
    return nc


def _legalize_waits(nc):
    """Walrus ISA structs have at most 1-2 sync-wait slots per instruction,
    but Tile's wait assignment can emit more. Move excess waits onto
    same-engine NoOps inserted immediately before the offending instruction
    (engines execute their stream in order, so an earlier wait on the same
    engine is equivalent)."""
    import bass_rust
    nop_i = [0]
    for f in nc.m.functions:
        for b in f.blocks:
            insts = b.instructions
            out = []
            changed = False
            for ins in insts:
                si = getattr(ins, "sync_info", None)
                waits = list(si.on_wait) if (si and si.on_wait) else []
                if len(waits) > 1:
                    eng = ins.engine
                    for w in waits[:-1]:
                        n = bass_rust.InstNoOp(name=f"I-nopw-{nop_i[0]}")
                        nop_i[0] += 1
                        n.engine = eng
                        n.sync_info = bass_rust.SyncInfo(
                            on_wait=[w], on_update=[])
                        out.append(n)
                    ins.sync_info = bass_rust.SyncInfo(
                        on_wait=[waits[-1]], on_update=list(si.on_update or []))
                    changed = True
                out.append(ins)
            if changed:
                b.instructions = out


def _prepare(inputs):
    """Host-side folding: per-channel vectors + fused/packed weights."""
    f = lambda k: np.asarray(inputs[k], np.float64)
    alpha = f("alpha").reshape(C)

    s_r = f("bn_rgb_w") / np.sqrt(f("bn_rgb_var") + EPS)
    t_r = f("bn_rgb_b") - f("bn_rgb_mean") * s_r
    s_d = f("bn_depth_w") / np.sqrt(f("bn_depth_var") + EPS)
    t_d = f("bn_depth_b") - f("bn_depth_mean") * s_d

    w_r = np.asarray(inputs["bn_rgb_w"], np.float32)
    w_d = np.asarray(inputs["bn_depth_w"], np.float32)
    idx_r = np.argsort(np.abs(w_r), kind="stable")[:K_EX]
    idx_d = np.argsort(np.abs(w_d), kind="stable")[:K_EX]
    mask_r = np.zeros(C, bool)
    mask_r[idx_r] = True
    mask_d = np.zeros(C, bool)
    mask_d[idx_d] = True

    A1 = np.where(mask_r, alpha * s_r, s_r)
    A2 = np.where(mask_r, (1 - alpha) * s_d, 0.0)
    A3 = np.where(mask_r, alpha * t_r + (1 - alpha) * t_d, t_r)
    D1 = np.where(mask_d, alpha * s_d, s_d)
    D2 = np.where(mask_d, (1 - alpha) * s_r, 0.0)
    D3 = np.where(mask_d, alpha * t_d + (1 - alpha) * t_r, t_d)

    qkv_w = f("qkv_w")
    Wv = qkv_w[2 * C :, :]
    Wc = f("proj_w") @ Wv
    w1, b1 = f("norm1_w"), f("norm1_b")
    Wc_f = Wc * w1[None, :]
    pb = f("proj_b") + Wc @ b1
    wc_rowsum = Wc_f.sum(axis=1)

    w2, b2 = f("norm2_w"), f("norm2_b")
    fc1_f = f("fc1_w") * w2[None, :]
    fb1 = f("fc1_b") + f("fc1_w") @ b2
    fc1_rowsum = fc1_f.sum(axis=1)
    fc2_w = f("fc2_w")
    fc2_b = f("fc2_b")
    wfh = 0.5 * f("normf_w")
    bf_ = f("normf_b")

    bf16 = ml_dtypes.bfloat16

    def pack_lhsT(wT, kt, m):
        # wT: [kt*128, m]  ->  [128, kt*m] with [p, k*m + col] = wT[128k+p, col]
        return np.ascontiguousarray(
            wT.reshape(kt, 128, m).transpose(1, 0, 2).reshape(128, kt * m))

    wc_pack = pack_lhsT(np.ascontiguousarray(Wc_f.T), CT, C).astype(bf16)
    fc1_pack = pack_lhsT(np.ascontiguousarray(fc1_f.T), CT, MLP).astype(bf16)
    fc2_pack = pack_lhsT(np.ascontiguousarray(fc2_w.T), MT, C).astype(bf16)

    vv = [A1, A2, A3, D1, D2, D3, pb, fc2_b, wfh, bf_]
    vecs = np.stack(vv, axis=-1).astype(np.float32)          # [C, NV]
    vecs = vecs.reshape(CT, 128, NV).transpose(1, 0, 2).reshape(128, CT * NV)
    vecs = np.ascontiguousarray(vecs)
    fb1_pack = np.ascontiguousarray(
        fb1.astype(np.float32).reshape(MT, 128).T)           # [128, MT]

    return {
        "wc": wc_pack,
        "fc1": fc1_pack,
        "fc2": fc2_pack,
        "vecs": vecs,
        "fb1": fb1_pack,
        "wcsum": (-wc_rowsum / np.sqrt(C)).astype(bf16).reshape(1, C),
        "fc1sum": (-fc1_rowsum / np.sqrt(C)).astype(bf16).reshape(1, MLP),
        "ident": np.eye(128, dtype=np.float32),
    }


def kernel(**inputs) -> np.ndarray:
    rgb = np.ascontiguousarray(np.asarray(inputs["rgb"], np.float32))
    dep = np.ascontiguousarray(np.asarray(inputs["depth"], np.float32))
    consts = _prepare(inputs)

    if "nc" not in _CACHE:
        _CACHE["nc"] = _build_nc()
    nc = _CACHE["nc"]

    in_maps = []
    for i in range(N_CORES):
        m = {
            "rgb": rgb[i * B_CORE : (i + 1) * B_CORE].reshape(ROWS, C),
            "dep": dep[i * B_CORE : (i + 1) * B_CORE].reshape(ROWS, C),
        }
        m.update(consts)
        in_maps.append(m)

    trace = bool(int(os.environ.get("KERNEL_TRACE", "0")))
    res = run_bass_kernel_spmd(nc, in_maps, list(range(N_CORES)), trace=trace)
    _CACHE["last_results"] = res
    out = np.concatenate(
        [res.results[i]["out"].reshape(B_CORE, T, C) for i in range(N_CORES)],
        axis=0)
    return out


if __name__ == "__main__":
    rng = np.random.default_rng(0)
    demo = {
        "rgb": rng.standard_normal((B, T, C), np.float32),
        "depth": rng.standard_normal((B, T, C), np.float32),
    }
    print("built module ok" if _build_nc() else "")
